# revision 24
# baseline (speedup 1.0000x reference)
"""Trainium2 Bass kernel for per-node multi-head neighbor attention.

Model (per node): Q = agent @ Wq + bq            (64 = 8 heads x 8 dv)
                  K = nbr @ Wk (+bk, cancels)    (16 x 64)
                  V = nbr @ Wv (+bv, folded)     (16 x 64)
                  att = softmax_k(Q.K/sqrt(8))   (8 x 16)
                  out = relu((att.V) @ Wo + bo') (64)

Sharding: (B,N) flattened to 80000 nodes, 10000 contiguous nodes per core,
weights replicated on all 8 cores.

Math notes exploited on-chip:
  - bk shifts all scores of an (n,h) group equally -> cancels in softmax.
  - sum_k att = 1, so ctx = att@(V0+bv) = att@V0 + bv; bv@Wo is folded into
    the output bias host-side (bo2 = bv @ Wo + bo).
  - softmax computed without max subtraction (scores bounded ~|2|), exp sums
    are normalized after the k-reduction to save a wide elementwise pass.

Scheduling notes: walrus allows a single sync-wait on PE matmul/ldweights, so
all constants arrive in ONE DMA (one sem lane) absorbed by a dummy transpose,
and the two PSUM pools chain allocations in PE program order so each matmul
has at most one fresh foreign-engine dependency:
  psN (4 banks): at(i) -> nbt(i) -> s(i) -> at(i+1)
  psP (4 banks): qt(i) -> kvt(i) -> attb(i) -> invb(i) -> o(i) -> qt(i+1)
"""

import sys
import numpy as np

sys.path.insert(0, "/opt/trn_rl_repo")

from contextlib import ExitStack

import bass_rust
import concourse.bass as bass
import concourse.bacc as bacc
import concourse.tile as tile
from concourse.tile import add_dep_helper
from concourse import mybir
from concourse.bass_utils import run_bass_kernel_spmd

FP32 = mybir.dt.float32

D_IN, DV, NHEAD, DOUT = 128, 8, 8, 64
DH = DV * NHEAD  # 64
B, N, K = 4, 20000, 16
N_CORES = 8
NODES_PER_CORE = (B * N) // N_CORES  # 10000
TILE_N = 128  # nodes per tile
INV_SQRT_DV = 1.0 / np.sqrt(DV)

# consts packing: [128, CCOLS] single DMA
_C_ID = 0        # ident  [128, 128]
_C_WQ = 128      # wq     [128, 64]
_C_WK = 192      # wk     [128, 64]
_C_WV = 256      # wv     [128, 64]
_C_WO = 320      # wo     [64, 64]
_C_MH = 384      # maskh  [64, 8]
_C_M2 = 392      # mask2  [8, 64]
_C_BQ = 456      # bq     [1, 64]
_C_BO = 520      # bo2    [1, 64]
_C_ON = 584      # ones   [1, 128]
CCOLS = 712


def _strip_pe_self_waits(nc):
    """Drop PE-semaphore waits from PE matmuls.

    PE issues in-order and its PSUM writes drain through one in-order pipe, so
    a matmul's wait on PE's own semaphore (PSUM slot-reuse WAW) is redundant —
    and walrus only supports a single sync-wait on the lowered LDWEIGHTS.
    """
    bad = []
    for blk in nc.m.functions[0].blocks:
        for i in blk.instructions:
            if type(i).__name__ != "InstMatmult":
                continue
            si = i.sync_info
            if not si or not si.on_wait:
                continue
            keep = [w for w in si.on_wait if not w.ant_name.startswith("PE")]
            if len(keep) != len(si.on_wait):
                i.sync_info = bass_rust.SyncInfo(on_wait=keep, on_update=si.on_update)
            if len(keep) > 1:
                bad.append((i.name, [(w.ant_name, w.wait_value) for w in keep]))
    if bad:
        raise RuntimeError(f"matmuls with >1 foreign wait (walrus limit): {bad}")


def build_program(nodes_per_core: int, repeats: int = 1):
    """Emit the Bass/Tile program for one core processing `nodes_per_core` nodes.

    repeats > 1 re-emits the whole pipeline (for dispatch-overhead-free timing).
    """
    nc = bacc.Bacc("TRN2")

    agent = nc.declare_dram_parameter("agent", [nodes_per_core, D_IN], FP32, isOutput=False)
    nbr = nc.declare_dram_parameter("nbr", [nodes_per_core * K, D_IN], FP32, isOutput=False)
    consts = nc.declare_dram_parameter("consts", [128, CCOLS], FP32, isOutput=False)
    out_d = nc.declare_dram_parameter("out", [nodes_per_core, DOUT], FP32, isOutput=True)
    att_d = nc.declare_dram_parameter("att", [nodes_per_core, NHEAD, K], FP32, isOutput=True)

    n_full = nodes_per_core // TILE_N
    tail = nodes_per_core - n_full * TILE_N
    tiles = [TILE_N] * n_full + ([tail] if tail else [])
    assert tail % 8 == 0, "tail must be a multiple of 8 nodes (whole 128-row chunks)"

    with tile.TileContext(nc) as tc, ExitStack() as ctx:
        singles = ctx.enter_context(tc.tile_pool(name="singles", bufs=1))
        io_pool = ctx.enter_context(tc.tile_pool(name="io", bufs=3))
        big = ctx.enter_context(tc.tile_pool(name="big", bufs=2))
        med = ctx.enter_context(tc.tile_pool(name="med", bufs=2))
        small = ctx.enter_context(tc.tile_pool(name="small", bufs=3))
        psN = ctx.enter_context(tc.tile_pool(name="psN", bufs=1, space="PSUM"))
        psP = ctx.enter_context(tc.tile_pool(name="psP", bufs=1, space="PSUM"))

        cs = singles.tile([128, CCOLS], FP32, tag="consts")
        nc.sync.dma_start(out=cs[:], in_=consts[:])
        id_sb = cs[:, _C_ID:_C_ID + 128]
        wq_sb = cs[:, _C_WQ:_C_WQ + DH]
        wk_sb = cs[:, _C_WK:_C_WK + DH]
        wv_sb = cs[:, _C_WV:_C_WV + DH]
        wo_sb = cs[0:DH, _C_WO:_C_WO + DOUT]
        maskh_sb = cs[0:DH, _C_MH:_C_MH + NHEAD]
        mask2_sb = cs[0:NHEAD, _C_M2:_C_M2 + DH]
        bq_sb = cs[0:1, _C_BQ:_C_BQ + DH]
        bo2_sb = cs[0:1, _C_BO:_C_BO + DOUT]
        ones_sb = cs[0:1, _C_ON:_C_ON + 128]

        # absorber: brings the consts-DMA tick into PE's clock (1 wait)
        dummy_ps = psN.tile([128, 128], FP32, tag="slot", name="dummy_ps")
        pe_state = [nc.tensor.transpose(dummy_ps[:], id_sb, id_sb)]

        def pe(inst):
            # pin PE order to program order so Tile's wait emission can elide
            # already-observed foreign ticks (walrus allows 1 wait per matmul)
            add_dep_helper(inst.ins, pe_state[0].ins, sync=False,
                           reason="PE program-order chain")
            pe_state[0] = inst
            return inst

        att_v = att_d.rearrange("n h k -> h n k")  # DMA view for att stores

        for _rep in range(repeats):
            emit_tiles(nc, tc, pe, tiles, agent, nbr, out_d, att_v,
                       io_pool, big, med, small, psN, psP,
                       id_sb, wq_sb, wk_sb, wv_sb, wo_sb, maskh_sb, mask2_sb,
                       bq_sb, bo2_sb, ones_sb)

    nc.compile()
    return nc


def emit_tiles(nc, tc, pe, tiles, agent, nbr, out_d, att_v,
               io_pool, big, med, small, psN, psP,
               id_sb, wq_sb, wk_sb, wv_sb, wo_sb, maskh_sb, mask2_sb,
               bq_sb, bo2_sb, ones_sb):
        n0 = 0
        for it, T in enumerate(tiles):
            KT_ = K * T          # (n,k) columns for this tile
            NCH = KT_ // 128     # 128-row chunks of neighbor data
            half = (NCH + 1) // 2
            nj = (KT_ + 511) // 512

            # ---- loads ----
            a_sb = io_pool.tile([TILE_N, D_IN], FP32, tag="a", name="a_sb")
            nc.sync.dma_start(out=a_sb[:T, :], in_=agent[n0:n0 + T, :])
            nb_sb = io_pool.tile([128, 16, 128], FP32, tag="nb", name="nb_sb")
            nb_view = nbr[n0 * K:(n0 + T) * K, :].rearrange("(c p) d -> p c d", p=128)
            nc.sync.dma_start(out=nb_sb[:, :NCH, :], in_=nb_view)

            # ---- agent transpose + Q projection ----
            at_ps = psN.tile([128, TILE_N], FP32, tag="slot", name="at_ps")
            pe(nc.tensor.transpose(at_ps[:, :T], a_sb[:T, :], id_sb[:T, :T]))
            at_sb = small.tile([128, TILE_N], FP32, tag="at_sb", name="at_sb")
            nc.scalar.copy(out=at_sb[:, :T], in_=at_ps[:, :T])

            qt_ps = psP.tile([DH, TILE_N], FP32, tag="slot", name="qt_ps")
            pe(nc.tensor.matmul(qt_ps[:, :T], wq_sb, at_sb[:, :T], start=True, stop=False))
            pe(nc.tensor.matmul(qt_ps[:, :T], bq_sb, ones_sb[:, :T], start=False, stop=True))
            qt_sb = small.tile([DH, TILE_N], FP32, tag="qt_sb", name="qt_sb")
            nc.scalar.copy(out=qt_sb[:, :T], in_=qt_ps[:, :T])

            # ---- neighbor transposes ----
            nbt_ps = psN.tile([128, 16, 128], FP32, tag="slot", name="nbt_ps")
            for c in range(NCH):
                pe(nc.tensor.transpose(nbt_ps[:, c, :], nb_sb[:, c, :], id_sb))
            nbt_sb = big.tile([128, 16, 128], FP32, tag="nbt_sb", name="nbt_sb")
            nc.scalar.copy(out=nbt_sb[:, :half, :], in_=nbt_ps[:, :half, :])
            if NCH > half:
                nc.vector.tensor_copy(out=nbt_sb[:, half:NCH, :], in_=nbt_ps[:, half:NCH, :])

            # ---- K,V projections: KVT[0:64]=K^T, KVT[64:128]=V^T ----
            kvt_ps = psP.tile([128, 16, 128], FP32, tag="slot", name="kvt_ps")
            nbt_flat = nbt_sb.rearrange("p c d -> p (c d)")
            kvt_flat = kvt_ps.rearrange("p c d -> p (c d)")
            for j in range(nj):
                j0, j1 = j * 512, min((j + 1) * 512, KT_)
                pe(nc.tensor.matmul(kvt_flat[0:DH, j0:j1], wk_sb, nbt_flat[:, j0:j1],
                                    start=True, stop=True))
            for j in range(nj):
                j0, j1 = j * 512, min((j + 1) * 512, KT_)
                pe(nc.tensor.matmul(kvt_flat[DH:128, j0:j1], wv_sb, nbt_flat[:, j0:j1],
                                    start=True, stop=True))
            kvt_sb = big.tile([128, 16, 128], FP32, tag="kvt_sb", name="kvt_sb")
            nc.scalar.copy(out=kvt_sb[:, :half, :], in_=kvt_ps[:, :half, :])
            if NCH > half:
                nc.vector.tensor_copy(out=kvt_sb[:, half:NCH, :], in_=kvt_ps[:, half:NCH, :])

            # ---- scores: P = K^T * broadcast(Q^T); S = maskh^T @ P ----
            kt3 = kvt_sb.rearrange("p c d -> p (c d)")[0:DH, :KT_].rearrange(
                "p (n k) -> p n k", k=K)
            qb = qt_sb[:, :T].unsqueeze(-1).broadcast_to([DH, T, K])
            p_sb = big.tile([DH, TILE_N, K], FP32, tag="p_sb", name="p_sb")
            nc.vector.tensor_mul(out=p_sb[:, :T, :], in0=kt3, in1=qb)

            s_ps = psN.tile([NHEAD, TILE_N, K], FP32, tag="slot", name="s_ps")
            p_flat = p_sb.rearrange("p n k -> p (n k)")
            s_flat = s_ps.rearrange("p n k -> p (n k)")
            for j in range(nj):
                j0, j1 = j * 512, min((j + 1) * 512, KT_)
                pe(nc.tensor.matmul(s_flat[:, j0:j1], maskh_sb, p_flat[:, j0:j1],
                                    start=True, stop=True))

            # ---- softmax (no max-sub); normalization deferred ----
            e_sb = med.tile([NHEAD, TILE_N, K], FP32, tag="e_sb", name="e_sb")
            nc.scalar.activation(out=e_sb[:, :T, :].rearrange("p n k -> p (n k)"),
                                 in_=s_flat[:, :KT_],
                                 func=mybir.ActivationFunctionType.Exp,
                                 scale=float(INV_SQRT_DV))
            esum = small.tile([NHEAD, TILE_N], FP32, tag="esum", name="esum")
            nc.vector.tensor_reduce(out=esum[:, :T], in_=e_sb[:, :T, :],
                                    axis=mybir.AxisListType.X, op=mybir.AluOpType.add)
            inv = small.tile([NHEAD, TILE_N], FP32, tag="inv", name="inv")
            nc.vector.reciprocal(out=inv[:, :T], in_=esum[:, :T])

            # ---- att = E * inv (gpsimd, SBUF only) and store ----
            att_sb = med.tile([NHEAD, TILE_N, K], FP32, tag="att_sb", name="att_sb")
            inv_b = inv[:, :T].unsqueeze(-1).broadcast_to([NHEAD, T, K])
            nc.gpsimd.tensor_mul(out=att_sb[:, :T, :], in0=e_sb[:, :T, :], in1=inv_b)
            nc.sync.dma_start(out=att_v[:, n0:n0 + T, :], in_=att_sb[:, :T, :])

            # ---- ctx: ATTB = mask2^T @ E ; PROD = ATTB * V^T ; reduce over k ----
            attb_ps = psP.tile([DH, TILE_N, K], FP32, tag="slot", name="attb_ps")
            attb_flat = attb_ps.rearrange("p n k -> p (n k)")
            e_flat = e_sb.rearrange("p n k -> p (n k)")
            for j in range(nj):
                j0, j1 = j * 512, min((j + 1) * 512, KT_)
                pe(nc.tensor.matmul(attb_flat[:, j0:j1], mask2_sb, e_flat[:, j0:j1],
                                    start=True, stop=True))
            vt3 = kvt_sb.rearrange("p c d -> p (c d)")[DH:128, :KT_].rearrange(
                "p (n k) -> p n k", k=K)
            prod_sb = big.tile([DH, TILE_N, K], FP32, tag="prod", name="prod_sb")
            nc.vector.tensor_mul(out=prod_sb[:, :T, :], in0=vt3, in1=attb_ps[:, :T, :])
            ctxu = small.tile([DH, TILE_N], FP32, tag="ctxu", name="ctxu")
            nc.vector.tensor_reduce(out=ctxu[:, :T], in_=prod_sb[:, :T, :],
                                    axis=mybir.AxisListType.X, op=mybir.AluOpType.add)

            # ---- normalize ctx: INVB = mask2^T @ inv ; ctxT = ctxu * INVB ----
            invb_ps = psP.tile([DH, TILE_N], FP32, tag="slot", name="invb_ps")
            pe(nc.tensor.matmul(invb_ps[:, :T], mask2_sb, inv[:, :T], start=True, stop=True))
            ctxt = small.tile([DH, TILE_N], FP32, tag="ctxt", name="ctxt")
            nc.vector.tensor_mul(out=ctxt[:, :T], in0=ctxu[:, :T], in1=invb_ps[:, :T])

            # ---- output projection + relu ----
            o_ps = psP.tile([TILE_N, DOUT], FP32, tag="slot", name="o_ps")
            pe(nc.tensor.matmul(o_ps[:T, :], ctxt[:, :T], wo_sb, start=True, stop=False))
            pe(nc.tensor.matmul(o_ps[:T, :], ones_sb[:, :T], bo2_sb, start=False, stop=True))
            o_sb = small.tile([TILE_N, DOUT], FP32, tag="o_sb", name="o_sb")
            nc.scalar.activation(out=o_sb[:T, :], in_=o_ps[:T, :],
                                 func=mybir.ActivationFunctionType.Relu)
            nc.sync.dma_start(out=out_d[n0:n0 + T, :], in_=o_sb[:T, :])

            n0 += T


_NC_CACHE = {}


def _get_program(nodes_per_core: int):
    if nodes_per_core not in _NC_CACHE:
        _NC_CACHE[nodes_per_core] = build_program(nodes_per_core)
    return _NC_CACHE[nodes_per_core]


def _host_constants(Wq, bq, Wk, Wv, Wo, bo, bv):
    maskh = np.zeros((DH, NHEAD), np.float32)
    for hd in range(DH):
        maskh[hd, hd // DV] = 1.0
    c = np.zeros((128, CCOLS), np.float32)
    c[:, _C_ID:_C_ID + 128] = np.eye(128, dtype=np.float32)
    c[:, _C_WQ:_C_WQ + DH] = Wq
    c[:, _C_WK:_C_WK + DH] = Wk
    c[:, _C_WV:_C_WV + DH] = Wv
    c[0:DH, _C_WO:_C_WO + DOUT] = Wo
    c[0:DH, _C_MH:_C_MH + NHEAD] = maskh
    c[0:NHEAD, _C_M2:_C_M2 + DH] = maskh.T
    c[0, _C_BQ:_C_BQ + DH] = bq
    c[0, _C_BO:_C_BO + DOUT] = bv @ Wo + bo
    c[0, _C_ON:_C_ON + 128] = 1.0
    return {"consts": c}


def kernel(agent_feat, neighbor_feat, Wq, bq, Wk, bk, Wv, bv, Wo, bo,
           trace=False, _nodes_per_core=None):
    agent_feat = np.asarray(agent_feat, np.float32)
    neighbor_feat = np.asarray(neighbor_feat, np.float32)
    npcore = _nodes_per_core or NODES_PER_CORE

    nc = _get_program(npcore)
    consts = _host_constants(np.asarray(Wq, np.float32), np.asarray(bq, np.float32),
                             np.asarray(Wk, np.float32), np.asarray(Wv, np.float32),
                             np.asarray(Wo, np.float32), np.asarray(bo, np.float32),
                             np.asarray(bv, np.float32))
    # bk intentionally unused: it cancels in softmax (constant shift per (n,h)).

    a_flat = agent_feat.reshape(B * N, D_IN)
    nb_flat = neighbor_feat.reshape(B * N * K, D_IN)
    in_maps = []
    for i in range(N_CORES):
        lo, hi = i * npcore, (i + 1) * npcore
        m = dict(consts)
        m["agent"] = np.ascontiguousarray(a_flat[lo:hi])
        m["nbr"] = np.ascontiguousarray(nb_flat[lo * K:hi * K])
        in_maps.append(m)

    res = run_bass_kernel_spmd(nc, in_maps, list(range(N_CORES)), trace=trace)
    out = np.concatenate([res.results[i]["out"] for i in range(N_CORES)], axis=0)
    att = np.concatenate([res.results[i]["att"] for i in range(N_CORES)], axis=0)
    out = out.reshape(B, N, DOUT)
    att = att.reshape(B, N, NHEAD, K)
    if trace:
        kernel.last_exec_time_ns = res.exec_time_ns
    return out, att



def timed_run(nc, in_maps, n_cores=N_CORES, iters=5, pipeline_iters=0):
    """Execute the prebuilt program on device-resident inputs, return
    (per-core results, list of wall-times per call in seconds)."""
    import time
    import jax
    from jax.sharding import Mesh, PartitionSpec, NamedSharding
    from jax.experimental.shard_map import shard_map
    from concourse import bass2jax, mybir as mb
    from concourse.bass2jax import _bass_exec_p, install_neuronx_cc_hook

    install_neuronx_cc_hook()
    partition_name = nc.partition_id_tensor.name if nc.partition_id_tensor else None

    in_names, out_names, out_avals, zero_outs = [], [], [], []
    for alloc in nc.m.functions[0].allocations:
        if not isinstance(alloc, mb.MemoryLocationSet):
            continue
        name = alloc.memorylocations[0].name
        if alloc.kind == "ExternalInput":
            if name != partition_name:
                in_names.append(name)
        elif alloc.kind == "ExternalOutput":
            out_names.append(name)
            shape = tuple(alloc.tensor_shape)
            dtype = mb.dt.np(alloc.dtype)
            out_avals.append(jax.core.ShapedArray(shape, dtype))
            zero_outs.append(np.zeros(shape, dtype))
    n_params = len(in_names)
    n_outs = len(out_avals)
    in_names.extend(out_names)
    if partition_name is not None:
        in_names.append(partition_name)
    donate = tuple(range(n_params, n_params + n_outs))

    def _body(*args):
        operands = list(args)
        if partition_name is not None:
            operands.append(bass2jax.partition_id_tensor())
        outs = _bass_exec_p.bind(
            *operands, out_avals=tuple(out_avals), in_names=tuple(in_names),
            out_names=tuple(out_names), lowering_input_output_aliases=(),
            sim_require_finite=True, sim_require_nnan=True, nc=nc)
        return tuple(outs)

    devices = jax.devices()[:n_cores]
    mesh = Mesh(np.asarray(devices), ("core",))
    in_specs = (PartitionSpec("core"),) * (n_params + n_outs)
    out_specs = (PartitionSpec("core"),) * len(out_names)
    # no donation: kernel writes every output element, and non-donated
    # buffers can be reused across timing iterations
    sharded = jax.jit(
        shard_map(_body, mesh=mesh, in_specs=in_specs, out_specs=out_specs,
                  check_rep=False),
        keep_unused=True)

    shard = NamedSharding(mesh, PartitionSpec("core"))
    concat_in = [
        jax.device_put(
            np.concatenate([np.asarray(in_maps[c][in_names[i]]) for c in range(n_cores)],
                           axis=0), shard)
        for i in range(n_params)
    ]
    jax.block_until_ready(concat_in)

    zeros_dev = [
        jax.device_put(np.zeros((n_cores * z.shape[0], *z.shape[1:]), z.dtype), shard)
        for z in zero_outs
    ]
    jax.block_until_ready(zeros_dev)
    out_arrs = sharded(*concat_in, *zeros_dev)  # warm-up / compile
    jax.block_until_ready(out_arrs)

    times = []
    for _ in range(iters):
        t0 = time.perf_counter()
        out_arrs = sharded(*concat_in, *zeros_dev)
        jax.block_until_ready(out_arrs)
        times.append(time.perf_counter() - t0)

    if pipeline_iters > 1:
        # queue many executions before blocking: per-exec time with dispatch
        # overhead amortized across the pipeline
        t0 = time.perf_counter()
        outs = [sharded(*concat_in, *zeros_dev) for _ in range(pipeline_iters)]
        jax.block_until_ready(outs)
        tq = time.perf_counter() - t0
        times.append(tq / pipeline_iters)

    results = [
        {name: np.asarray(out_arrs[i]).reshape(n_cores, *out_avals[i].shape)[c]
         for i, name in enumerate(out_names)}
        for c in range(n_cores)
    ]
    return results, times

kernel.last_exec_time_ns = None
